# revision 3
# baseline (speedup 1.0000x reference)
"""GQA attention block (QKV proj + causal attention + output proj) on 8 trn2 cores.

Sharding: core c -> (batch b = c//4, kv-group g = c%4). Each core computes 4 Q
heads (one KV-head group) of one batch and a partial o_proj output; the host
sums the 4 partials per batch (row-sharded o_proj all-reduce done host-side).

Matmul inputs are bf16 (1 cycle/row on the PE vs 4 for fp32); accumulation is
fp32 in PSUM. All device inputs are pre-tiled host-side to [128, ko, ...] so
every DMA is a full-bandwidth contiguous-per-partition transfer. Phase 1 runs
tcol-major with six concurrent full-depth PSUM accumulation groups, paced by
the streaming x^T DMA.

Attention (phase 2) computes transposed scores S^T[tk, tq] and then y^T[d, tq]
DIRECTLY via AV matmuls with V-natural stationary and S^T as the wide moving
operand (N=512), so no per-head output transposes are needed and every PE op
streams >=384 columns. The softmax denominator comes from per-partition
partial sums of exp(S^T) accumulated on the DVE/GpSimd engines (fp16), folded
to a per-query denominator broadcast across partitions by one ones-stationary
matmul per (head, chunk); normalization fuses into the PSUM->SBUF evacuation
of y^T. o_proj partials are emitted one 512-query chunk late so their matmuls
fill PE bubbles left by the Scalar engine's exp stream.
"""

import math

import numpy as np

# Model dims (hardcoded per contract; kernel.py must be self-contained).
B = 2
T = 2048
E = 2048
HD = 128               # head dim
NH = 16                # query heads total
NKV = 4                # kv heads total
NHC = 4                # query heads per core
P = 128
KO = E // P            # 16 contraction subtiles of 128
TQC = T // 512         # 4 query chunks of 512
TB = T // P            # 16 t blocks of 128
SCALE = 1.0 / math.sqrt(HD)
N_CORES = 8

_NC_CACHE = {}


def _build_nc(loop_n=1, ahead=2):
    import concourse.bacc as bacc
    import concourse.mybir as mybir
    import concourse.tile as tile
    from concourse.masks import make_identity, make_upper_triangular

    f32 = mybir.dt.float32
    bf16 = mybir.dt.bfloat16
    nc = bacc.Bacc(None, target_bir_lowering=False)

    # Inputs are host-pre-tiled: [128 partitions, ko, chunk] with the e
    # (contraction) axis split as e = ko*128 + p.
    xT3 = nc.dram_tensor("xT3", [P, KO, T], bf16, kind="ExternalInput")
    wqT3 = nc.dram_tensor("wqT3", [P, KO, NHC * HD], bf16, kind="ExternalInput")
    wkT3 = nc.dram_tensor("wkT3", [P, KO, HD], bf16, kind="ExternalInput")
    wvT3 = nc.dram_tensor("wvT3", [P, KO, HD], bf16, kind="ExternalInput")
    woT3 = nc.dram_tensor("woT3", [P, NHC, E], bf16, kind="ExternalInput")
    out = nc.dram_tensor("out", [T, E], bf16, kind="ExternalOutput")

    out_r = out.rearrange("(tb p) e -> p tb e", p=P)      # [128, 16, E]

    with tile.TileContext(nc) as tc:
        if loop_n > 1:
            # Bench-only: run the whole (idempotent) kernel body loop_n
            # times device-side so one NEFF execution measures steady-state
            # per-iteration device time.
            with tc.For_i(0, loop_n):
                _emit_body(nc, tc, mybir, tile, make_identity,
                           make_upper_triangular, f32, bf16,
                           xT3, wqT3, wkT3, wvT3, woT3, out_r, ahead)
        else:
            _emit_body(nc, tc, mybir, tile, make_identity,
                       make_upper_triangular, f32, bf16,
                       xT3, wqT3, wkT3, wvT3, woT3, out_r, ahead)

    nc.finalize()
    return nc


def _emit_body(nc, tc, mybir, tile, make_identity, make_upper_triangular,
               f32, bf16, xT3, wqT3, wkT3, wvT3, woT3, out_r, ahead=2):
    fp16 = mybir.dt.float16
    with (
        tc.tile_pool(name="const", bufs=1) as constp,
        tc.tile_pool(name="qkv", bufs=1) as qkvp,
    ):
        identity = constp.tile([P, P], bf16, tag="ident")
        make_identity(nc, identity)

        # tri[p, q] = 1.0 where p <= q — causal mask for the one
        # tk==tq diagonal 128x128 sub-block of S^T.
        tri = constp.tile([P, P], bf16, tag="tri")
        make_upper_triangular(nc, tri[:], val=1.0, diag=True)

        # all-ones stationary: one matmul against it reduces the fp16
        # denominator partials over partitions AND broadcasts the result
        # to every output partition.
        ones16 = constp.tile([P, P], fp16, tag="ones16")
        nc.vector.memset(ones16[:], 1.0)

        QT = qkvp.tile([P, NHC, T], bf16, tag="QT")    # q^T per head [d, t]
        KT = qkvp.tile([P, T], bf16, tag="KT")         # k^T [d, t]
        VT = qkvp.tile([P, T], bf16, tag="VT")         # v^T [d, t]
        VN = qkvp.tile([P, TB, HD], bf16, tag="VN")    # v natural blocks [tk, d]
        YT = qkvp.tile([P, NHC, T], bf16, tag="YT")    # y^T per head [d, t]
        WOT = qkvp.tile([P, NHC, E], bf16, tag="WOT")

        # ---- Phase 1: projections. q^T/k^T/v^T = W @ x^T, contracting
        # over e with full-depth (K=2048) PSUM accumulation, six output
        # chunks (K, V, Q0..Q3 for one tcol) in flight at once and the
        # e-subtile loop innermost so compute tracks the x^T DMA stream.
        with (
            tc.tile_pool(name="w1", bufs=1) as w1p,
            tc.tile_pool(name="ps1", bufs=1, space="PSUM") as ps1,
        ):
            XT = w1p.tile([P, KO, T], bf16, tag="XT")
            WQT = w1p.tile([P, KO, NHC * HD], bf16, tag="WQT")
            WKT = w1p.tile([P, KO, HD], bf16, tag="WKT")
            WVT = w1p.tile([P, KO, HD], bf16, tag="WVT")

            # DMA order sets the critical path: K/V weights and the first
            # Q-weight chunk, then x^T streamed per (ko, half-T) so the
            # first matmul starts ~5us in and compute tracks the stream.
            nc.sync.dma_start(WKT[:], wkT3[:])
            nc.sync.dma_start(WVT[:], wvT3[:])
            nc.sync.dma_start(WQT[:, 0:4], wqT3[:, 0:4])
            for ko in range(KO):
                nc.sync.dma_start(XT[:, ko, 0:1024], xT3[:, ko, 0:1024])
                if ko % 4 == 3 and ko < 12:
                    q = ko // 4 + 1
                    nc.sync.dma_start(
                        WQT[:, 4 * q:4 * (q + 1)], wqT3[:, 4 * q:4 * (q + 1)]
                    )
            for ko in range(KO):
                nc.sync.dma_start(XT[:, ko, 1024:2048], xT3[:, ko, 1024:2048])
            for h in range(NHC):
                nc.sync.dma_start(WOT[:, h], woT3[:, h])

            # Touch the Exp table now so the one-time activation-table
            # load happens during the x^T DMA stream, not at the first
            # real exp in phase 2.
            warm = w1p.tile([P, 1], f32, tag="warm")
            nc.scalar.activation(
                warm[:], WKT[:, 0, 0:1],
                mybir.ActivationFunctionType.Exp, scale=0.0,
            )

            # Keep the PE busy while x^T streams in: real (non-transpose)
            # matmuls engage the HAM activity monitor so the PE clock is
            # already at 8/8 when the projection matmuls start.
            for _ in range(60):
                pwm = ps1.tile([P, 64], f32, tag="ps_t", name="pwm", bufs=2)
                nc.tensor.matmul(pwm[:], identity[:], identity[:, 0:64],
                                 start=True, stop=True)

            def make_vn(tcol):
                # v^T -> v natural layout blocks for the AV stationary.
                for tb in range(4 * tcol, 4 * tcol + 4):
                    pst = ps1.tile([P, P], bf16, tag="ps_t", name="pst",
                                   bufs=2)
                    nc.tensor.transpose(
                        pst[:], VT[:, tb * P:(tb + 1) * P], identity[:]
                    )
                    nc.vector.tensor_copy(VN[:, tb], pst[:])

            for tcol in range(TQC):
                cols = slice(tcol * 512, (tcol + 1) * 512)
                psK = ps1.tile([P, 512], f32, tag="ps_proj", name="psK",
                               bufs=6)
                psV = ps1.tile([P, 512], f32, tag="ps_proj", name="psV",
                               bufs=6)
                psQ = [
                    ps1.tile([P, 512], f32, tag="ps_proj", name=f"psQ{h}",
                             bufs=6)
                    for h in range(NHC)
                ]
                for ko in range(KO):
                    st = ko == 0
                    sp = ko == KO - 1
                    xk = XT[:, ko, cols]
                    nc.tensor.matmul(psK[:], WKT[:, ko], xk, start=st, stop=sp)
                    nc.tensor.matmul(psV[:], WVT[:, ko], xk, start=st, stop=sp)
                    for h in range(NHC):
                        nc.tensor.matmul(
                            psQ[h][:], WQT[:, ko, h * HD:(h + 1) * HD], xk,
                            start=st, stop=sp,
                        )
                nc.vector.tensor_copy(KT[:, cols], psK[:])
                nc.vector.tensor_copy(VT[:, cols], psV[:])
                for h in range(NHC):
                    nc.vector.tensor_copy(QT[:, h, cols], psQ[h][:])
                make_vn(tcol)

        # ---- Phases 2+3: causal attention with y^T produced directly
        # (V-natural stationary, S^T moving), denominator on DVE/GpSimd,
        # and the o_proj partial for query chunk tqc-1 emitted after chunk
        # tqc's attention so its matmuls never wait on fresh YT.
        with (
            tc.tile_pool(name="work", bufs=1) as work,
            tc.tile_pool(name="ps2", bufs=1, space="PSUM") as ps2,
        ):
            def oproj_chunk(tqc):
                # out[t, e] = sum_h y_h^T.T @ woT_h for 4 t-blocks
                for tb in range(4 * tqc, 4 * tqc + 4):
                    for ec in range(4):
                        ps = ps2.tile([P, 512], f32, tag="pso", name="pso",
                                      bufs=2)
                        for h2 in range(NHC):
                            nc.tensor.matmul(
                                ps[:],
                                YT[:, h2, tb * P:(tb + 1) * P],
                                WOT[:, h2, ec * 512:(ec + 1) * 512],
                                start=(h2 == 0),
                                stop=(h2 == NHC - 1),
                            )
                        osb = work.tile([P, 512], bf16, tag="osb", name="osb",
                                        bufs=4)
                        nc.vector.tensor_copy(osb[:], ps[:])
                        nc.sync.dma_start(
                            out_r[:, tb, ec * 512:(ec + 1) * 512], osb[:]
                        )

            for tqc in range(TQC):
                ntk = 4 * (tqc + 1)   # tk blocks up to the diagonal
                # gpsimd takes the first g (earliest-ready, full-width)
                # denominator blocks; DVE the rest. g=0 for tqc=0 because
                # there the first DVE block must be the full-width one.
                g = (ntk // 3) if tqc > 0 else 0
                for h in range(NHC):

                    def scores_exp(tk):
                        # S^T[tk, tq] for the causally-valid tq columns,
                        # exp'd into bf16; the single diagonal 128x128
                        # sub-block gets the triangular mask.
                        i = tk - 4 * tqc
                        off = max(0, i) * P
                        w = 512 - off
                        pss = ps2.tile([P, 512], f32, tag="pss", name="pss",
                                       bufs=3)
                        nc.tensor.matmul(
                            pss[:, 0:w],
                            KT[:, tk * P:(tk + 1) * P],
                            QT[:, h, tqc * 512 + off:(tqc + 1) * 512],
                            start=True,
                            stop=True,
                        )
                        es = work.tile([P, 512], bf16, tag="es", name="es",
                                       bufs=6)
                        nc.scalar.activation(
                            es[:, 0:w], pss[:, 0:w],
                            mybir.ActivationFunctionType.Exp,
                            scale=SCALE,
                        )
                        if i >= 0:
                            nc.vector.tensor_tensor(
                                out=es[:, 0:P], in0=es[:, 0:P], in1=tri[:],
                                op=mybir.AluOpType.mult,
                            )
                        return es

                    psy = ps2.tile([P, 512], f32, tag="psy", name="psy",
                                   bufs=2)
                    dA = work.tile([P, 512], fp16, tag="dA", name="dA",
                                   bufs=2)
                    dB = None
                    if g > 0:
                        dB = work.tile([P, 512], fp16, tag="dB", name="dB",
                                       bufs=2)

                    pipe = {}
                    for tk in range(min(ahead, ntk)):
                        pipe[tk] = scores_exp(tk)
                    for tk in range(ntk):
                        if tk + ahead < ntk:
                            pipe[tk + ahead] = scores_exp(tk + ahead)
                        i = tk - 4 * tqc
                        off = max(0, i) * P
                        w = 512 - off
                        es = pipe.pop(tk)
                        # AV: y^T[d, tq] += V_nat[tk].T @ S^T[tk, tq]
                        nc.tensor.matmul(
                            psy[:, off:512],
                            VN[:, tk],
                            es[:, 0:w],
                            start=(tk == 0),
                            stop=(tk == ntk - 1),
                        )
                        # Denominator partials: d[p, tq] += es[p, tq]
                        if tk < g:
                            eng, acc, first = nc.gpsimd, dB, tk == 0
                        else:
                            eng, acc, first = nc.vector, dA, tk == g
                        if first:
                            eng.tensor_copy(acc[:, off:512], es[:, 0:w])
                        else:
                            eng.tensor_tensor(
                                out=acc[:, off:512], in0=acc[:, off:512],
                                in1=es[:, 0:w], op=mybir.AluOpType.add,
                            )

                    # den[tq] = sum_p d[p, tq], broadcast to all partitions
                    # by the ones-stationary matmul; then y^T / den fused
                    # into the PSUM evacuation.
                    psden = ps2.tile([P, 512], f32, tag="psden", name="psden",
                                     bufs=1)
                    nc.tensor.matmul(psden[:], ones16[:], dA[:],
                                     start=True, stop=(g == 0))
                    if g > 0:
                        nc.tensor.matmul(psden[:], ones16[:], dB[:],
                                         start=False, stop=True)
                    rden = work.tile([P, 512], f32, tag="rden", name="rden",
                                     bufs=2)
                    nc.vector.reciprocal(rden[:], psden[:])
                    nc.vector.tensor_tensor(
                        out=YT[:, h, tqc * 512:(tqc + 1) * 512],
                        in0=psy[:], in1=rden[:],
                        op=mybir.AluOpType.mult,
                    )

                if tqc > 0:
                    oproj_chunk(tqc - 1)
            oproj_chunk(TQC - 1)


def _get_nc():
    if "nc" not in _NC_CACHE:
        _NC_CACHE["nc"] = _build_nc()
    return _NC_CACHE["nc"]


def _tile_e(arr, chunk):
    # [out_dim, E] -> transpose -> [E, out_dim] -> [128, KO, out_dim]
    import ml_dtypes

    t = np.ascontiguousarray(arr.T)
    t = t.reshape(KO, P, chunk).transpose(1, 0, 2)
    return np.ascontiguousarray(t).astype(ml_dtypes.bfloat16)


def _in_maps(x, wq, wk, wv, wo):
    import ml_dtypes

    bf16 = ml_dtypes.bfloat16
    xT3 = [_tile_e(x[b], T) for b in range(B)]
    wqT3 = [_tile_e(wq[g * 512:(g + 1) * 512], 512) for g in range(NKV)]
    wkT3 = [_tile_e(wk[g * HD:(g + 1) * HD], HD) for g in range(NKV)]
    wvT3 = [_tile_e(wv[g * HD:(g + 1) * HD], HD) for g in range(NKV)]
    # wo columns for group g, transposed to [512, E] then tiled to [128,4,E]
    woT3 = []
    for g in range(NKV):
        t = np.ascontiguousarray(wo[:, g * 512:(g + 1) * 512].T)  # [512, E]
        t = t.reshape(NHC, P, E).transpose(1, 0, 2)
        woT3.append(np.ascontiguousarray(t).astype(bf16))
    maps = []
    for c in range(N_CORES):
        b, g = divmod(c, NKV)
        maps.append({
            "xT3": xT3[b],
            "wqT3": wqT3[g],
            "wkT3": wkT3[g],
            "wvT3": wvT3[g],
            "woT3": woT3[g],
        })
    return maps


def kernel(x, wq, wk, wv, wo):
    from concourse.bass_utils import run_bass_kernel_spmd

    x = np.asarray(x, dtype=np.float32)
    wq = np.asarray(wq, dtype=np.float32)
    wk = np.asarray(wk, dtype=np.float32)
    wv = np.asarray(wv, dtype=np.float32)
    wo = np.asarray(wo, dtype=np.float32)

    nc = _get_nc()
    in_maps = _in_maps(x, wq, wk, wv, wo)

    res = run_bass_kernel_spmd(nc, in_maps, core_ids=list(range(N_CORES)))

    partials = [np.asarray(res.results[c]["out"]).astype(np.float32)
                for c in range(N_CORES)]
    out = np.empty((B, T, E), dtype=np.float32)
    for b in range(B):
        acc = partials[NKV * b]
        for g in range(1, NKV):
            acc = acc + partials[NKV * b + g]
        out[b] = acc
    return out


# revision 36
# speedup vs baseline: 1.1292x; 1.1292x over previous
"""GQA attention block (QKV proj + causal attention + output proj) on 8 trn2 cores.

Sharding: core c -> (batch b = c//4, kv-group g = c%4). Each core computes 4 Q
heads (one KV-head group) of one batch and a partial o_proj output; the host
sums the 4 partials per batch (row-sharded o_proj all-reduce done host-side).

Matmul inputs are bf16 (1 cycle/row on the PE vs 4 for fp32); accumulation is
fp32 in PSUM. All device inputs are pre-tiled host-side to [128, ko, ...] so
every DMA is a full-bandwidth contiguous-per-partition transfer. Phase 1 runs
tcol-major with six concurrent full-depth PSUM accumulation groups, paced by
the streaming x^T DMA.

Attention (phase 2) computes transposed scores S^T[tk, tq] and then y^T[d, tq]
DIRECTLY via AV matmuls with V-natural stationary and S^T as the wide moving
operand (N=512), so no per-head output transposes are needed and every PE op
streams >=384 columns. The softmax denominator comes from per-partition
partial sums of exp(S^T) accumulated on the DVE (fp16; GpSimd measured 3x
slower than its cost model — do not use), folded to a per-query denominator
broadcast across partitions by one ones-stationary matmul per (head, chunk);
normalization fuses into the PSUM->SBUF evacuation of y^T. o_proj partials
are emitted one 512-query chunk late so their matmuls fill PE bubbles left
by the Scalar engine's exp stream.

HW notes (measured on the axon trn2 cores): per-dma_start + per-descriptor
overhead dominates DRAM writes — the o_proj output leaves in 16 DMAs of
[128, 2048] (4KB per partition per descriptor), 3x faster than 512-wide
chunks. x^T input stays split in half-T chunks: merging to full-T starves
tcol 0/1 (model and HW agree, +40us HW). fp8(e4m3) anywhere busts the 2e-2
gate (measured 2.5e-2..4e-2 on CPU) — everything stays bf16.
"""

import math

import numpy as np

# Model dims (hardcoded per contract; kernel.py must be self-contained).
B = 2
T = 2048
E = 2048
HD = 128               # head dim
NH = 16                # query heads total
NKV = 4                # kv heads total
NHC = 4                # query heads per core
P = 128
KO = E // P            # 16 contraction subtiles of 128
TQC = T // 512         # 4 query chunks of 512
TB = T // P            # 16 t blocks of 128
SCALE = 1.0 / math.sqrt(HD)
N_CORES = 8

_NC_CACHE = {}


def _build_nc(loop_n=1, ahead=2, gp=False, do_attn=True, do_oproj=True,
              do_outdma=True, osb_act=0, p1act=1, stagger=1, xmerge=0):
    import concourse.bacc as bacc
    import concourse.mybir as mybir
    import concourse.tile as tile
    from concourse.masks import make_identity, make_upper_triangular

    f32 = mybir.dt.float32
    bf16 = mybir.dt.bfloat16
    nc = bacc.Bacc(None, target_bir_lowering=False)

    # Inputs are host-pre-tiled: [128 partitions, ko, chunk] with the e
    # (contraction) axis split as e = ko*128 + p.
    xT3 = nc.dram_tensor("xT3", [P, KO, T], bf16, kind="ExternalInput")
    wqT3 = nc.dram_tensor("wqT3", [P, KO, NHC * HD], bf16, kind="ExternalInput")
    wkT3 = nc.dram_tensor("wkT3", [P, KO, HD], bf16, kind="ExternalInput")
    wvT3 = nc.dram_tensor("wvT3", [P, KO, HD], bf16, kind="ExternalInput")
    woT3 = nc.dram_tensor("woT3", [P, NHC, E], bf16, kind="ExternalInput")
    out = nc.dram_tensor("out", [T, E], bf16, kind="ExternalOutput")

    out_r = out.rearrange("(tb p) e -> p tb e", p=P)      # [128, 16, E]

    with tile.TileContext(nc) as tc:
        if loop_n > 1:
            # Bench-only: run the whole (idempotent) kernel body loop_n
            # times device-side so one NEFF execution measures steady-state
            # per-iteration device time.
            with tc.For_i(0, loop_n):
                _emit_body(nc, tc, mybir, tile, make_identity,
                           make_upper_triangular, f32, bf16,
                           xT3, wqT3, wkT3, wvT3, woT3, out_r, ahead, gp,
                           do_attn, do_oproj, do_outdma, osb_act, p1act, stagger, xmerge)
        else:
            _emit_body(nc, tc, mybir, tile, make_identity,
                       make_upper_triangular, f32, bf16,
                       xT3, wqT3, wkT3, wvT3, woT3, out_r, ahead, gp,
                       do_attn, do_oproj, do_outdma, osb_act, p1act, stagger, xmerge)

    nc.finalize()
    return nc


def _emit_body(nc, tc, mybir, tile, make_identity, make_upper_triangular,
               f32, bf16, xT3, wqT3, wkT3, wvT3, woT3, out_r, ahead=2,
               gp=False, do_attn=True, do_oproj=True, do_outdma=True,
               osb_act=0, p1act=1, stagger=1, xmerge=0):
    fp16 = mybir.dt.float16
    with (
        tc.tile_pool(name="const", bufs=1) as constp,
        tc.tile_pool(name="qkv", bufs=1) as qkvp,
    ):
        identity = constp.tile([P, P], bf16, tag="ident")
        make_identity(nc, identity)

        # tri[p, q] = 1.0 where p <= q — causal mask for the one
        # tk==tq diagonal 128x128 sub-block of S^T.
        tri = constp.tile([P, P], bf16, tag="tri")
        make_upper_triangular(nc, tri[:], val=1.0, diag=True)

        # all-ones stationary: one matmul against it reduces the fp16
        # denominator partials over partitions AND broadcasts the result
        # to every output partition.
        ones16 = constp.tile([P, P], fp16, tag="ones16")
        nc.vector.memset(ones16[:], 1.0)

        QT = qkvp.tile([P, NHC, T], bf16, tag="QT")    # q^T per head [d, t]
        KT = qkvp.tile([P, T], bf16, tag="KT")         # k^T [d, t]
        VT = qkvp.tile([P, T], bf16, tag="VT")         # v^T [d, t]
        VN = qkvp.tile([P, TB, HD], bf16, tag="VN")    # v natural blocks [tk, d]
        YT = qkvp.tile([P, NHC, T], bf16, tag="YT")    # y^T per head [d, t]
        WOT = qkvp.tile([P, NHC, E], bf16, tag="WOT")

        # ---- Phase 1: projections. q^T/k^T/v^T = W @ x^T, contracting
        # over e with full-depth (K=2048) PSUM accumulation, six output
        # chunks (K, V, Q0..Q3 for one tcol) in flight at once and the
        # e-subtile loop innermost so compute tracks the x^T DMA stream.
        with (
            tc.tile_pool(name="w1", bufs=1) as w1p,
            tc.tile_pool(name="ps1", bufs=1, space="PSUM") as ps1,
        ):
            XT = w1p.tile([P, KO, T], bf16, tag="XT")
            WQT = w1p.tile([P, KO, NHC * HD], bf16, tag="WQT")
            WKT = w1p.tile([P, KO, HD], bf16, tag="WKT")
            WVT = w1p.tile([P, KO, HD], bf16, tag="WVT")

            # DMA order sets the critical path: K/V weights and the first
            # Q-weight chunk, then x^T streamed per (ko, half-T) so the
            # first matmul starts ~5us in and compute tracks the stream.
            nc.sync.dma_start(WKT[:], wkT3[:])
            nc.sync.dma_start(WVT[:], wvT3[:])
            nc.sync.dma_start(WQT[:, 0:4], wqT3[:, 0:4])
            for ko in range(KO):
                if xmerge:
                    nc.sync.dma_start(XT[:, ko], xT3[:, ko])
                else:
                    nc.sync.dma_start(XT[:, ko, 0:1024], xT3[:, ko, 0:1024])
                if ko % 4 == 3 and ko < 12:
                    q = ko // 4 + 1
                    nc.sync.dma_start(
                        WQT[:, 4 * q:4 * (q + 1)], wqT3[:, 4 * q:4 * (q + 1)]
                    )
            if not xmerge:
                for ko in range(KO):
                    nc.sync.dma_start(XT[:, ko, 1024:2048],
                                      xT3[:, ko, 1024:2048])
            for h in range(NHC):
                nc.sync.dma_start(WOT[:, h], woT3[:, h])

            # Touch the Exp table now so the one-time activation-table
            # load happens during the x^T DMA stream, not at the first
            # real exp in phase 2.
            warm = w1p.tile([P, 1], f32, tag="warm")
            nc.scalar.activation(
                warm[:], WKT[:, 0, 0:1],
                mybir.ActivationFunctionType.Exp, scale=0.0,
            )

            # Keep the PE busy while x^T streams in: real (non-transpose)
            # matmuls engage the HAM activity monitor so the PE clock is
            # already at 8/8 when the projection matmuls start.
            for _ in range(60):
                pwm = ps1.tile([P, 64], f32, tag="ps_t", name="pwm", bufs=2)
                nc.tensor.matmul(pwm[:], identity[:], identity[:, 0:64],
                                 start=True, stop=True)

            def make_vn(tcol):
                # v^T -> v natural layout blocks for the AV stationary.
                for tb in range(4 * tcol, 4 * tcol + 4):
                    pst = ps1.tile([P, P], bf16, tag="ps_t", name="pst",
                                   bufs=2)
                    nc.tensor.transpose(
                        pst[:], VT[:, tb * P:(tb + 1) * P], identity[:]
                    )
                    nc.vector.tensor_copy(VN[:, tb], pst[:])

            for tcol in range(TQC):
                cols = slice(tcol * 512, (tcol + 1) * 512)
                psK = ps1.tile([P, 512], f32, tag="ps_proj", name="psK",
                               bufs=6)
                psV = ps1.tile([P, 512], f32, tag="ps_proj", name="psV",
                               bufs=6)
                psQ = [
                    ps1.tile([P, 512], f32, tag="ps_proj", name=f"psQ{h}",
                             bufs=6)
                    for h in range(NHC)
                ]
                if tcol == 0 or not stagger:
                    # x^T is still streaming in: interleave all six
                    # accumulation groups per ko so compute tracks the DMA.
                    for ko in range(KO):
                        st = ko == 0
                        sp = ko == KO - 1
                        xk = XT[:, ko, cols]
                        nc.tensor.matmul(psK[:], WKT[:, ko], xk,
                                         start=st, stop=sp)
                        nc.tensor.matmul(psV[:], WVT[:, ko], xk,
                                         start=st, stop=sp)
                        for h in range(NHC):
                            nc.tensor.matmul(
                                psQ[h][:], WQT[:, ko, h * HD:(h + 1) * HD],
                                xk, start=st, stop=sp,
                            )
                else:
                    # x^T resident: run each accumulation group to
                    # completion so the six PSUM tiles free at staggered
                    # times and the next tcol never waits on evacuation.
                    for ps, wsl in (
                        (psV, WVT[:, :, 0:HD]),
                        (psK, WKT[:, :, 0:HD]),
                        *((psQ[h], WQT[:, :, h * HD:(h + 1) * HD])
                          for h in range(NHC)),
                    ):
                        for ko in range(KO):
                            nc.tensor.matmul(
                                ps[:], wsl[:, ko], XT[:, ko, cols],
                                start=(ko == 0), stop=(ko == KO - 1),
                            )
                p1copy = nc.scalar.copy if p1act else nc.vector.tensor_copy
                p1copy(VT[:, cols], psV[:])
                if tcol == TQC - 1:
                    # transposes right after the V copy so they overlap the
                    # remaining K/Q evacuations ahead of the pool barrier.
                    make_vn(tcol)
                p1copy(KT[:, cols], psK[:])
                for h in range(NHC):
                    p1copy(QT[:, h, cols], psQ[h][:])
                if tcol < TQC - 1:
                    make_vn(tcol)

        # ---- Phases 2+3: causal attention with y^T produced directly
        # (V-natural stationary, S^T moving), denominator on DVE/GpSimd,
        # and the o_proj partial for query chunk tqc-1 emitted after chunk
        # tqc's attention so its matmuls never wait on fresh YT.
        with (
            tc.tile_pool(name="work", bufs=1) as work,
            tc.tile_pool(name="ps2", bufs=1, space="PSUM") as ps2,
        ):
            def oproj_chunk(tqc):
                # out[t, e] = sum_h y_h^T.T @ woT_h for 4 t-blocks; the
                # four 512-wide strips stage into one [128, 2048] tile so
                # each t-block leaves in a single 4KB-per-partition DMA
                # (per-descriptor cost dominates small DRAM writes).
                for tb in range(4 * tqc, 4 * tqc + 4):
                    ost = work.tile([P, E], bf16, tag="osb", name="ost",
                                    bufs=2)
                    for ec in range(4):
                        ps = ps2.tile([P, 512], f32, tag="pso", name="pso",
                                      bufs=(2 if ahead < 3 else 1))
                        for h2 in range(NHC):
                            nc.tensor.matmul(
                                ps[:],
                                YT[:, h2, tb * P:(tb + 1) * P],
                                WOT[:, h2, ec * 512:(ec + 1) * 512],
                                start=(h2 == 0),
                                stop=(h2 == NHC - 1),
                            )
                        if osb_act and (tb * 4 + ec) % 2:
                            nc.scalar.copy(
                                ost[:, ec * 512:(ec + 1) * 512], ps[:]
                            )
                        else:
                            nc.vector.tensor_copy(
                                ost[:, ec * 512:(ec + 1) * 512], ps[:]
                            )
                    if do_outdma:
                        nc.sync.dma_start(out_r[:, tb], ost[:])

            for tqc in range(TQC if do_attn else 0):
                ntk = 4 * (tqc + 1)   # tk blocks up to the diagonal
                # gpsimd takes the first g (earliest-ready, full-width)
                # denominator blocks; DVE the rest. g=0 for tqc=0 because
                # there the first DVE block must be the full-width one.
                g = (ntk // 3) if (tqc > 0 and gp) else 0
                for h in range(NHC):

                    def scores_exp(tk):
                        # S^T[tk, tq] for the causally-valid tq columns,
                        # exp'd into bf16; the single diagonal 128x128
                        # sub-block gets the triangular mask.
                        i = tk - 4 * tqc
                        off = max(0, i) * P
                        w = 512 - off
                        pss = ps2.tile([P, 512], f32, tag="pss", name="pss",
                                       bufs=ahead + 1)
                        nc.tensor.matmul(
                            pss[:, 0:w],
                            KT[:, tk * P:(tk + 1) * P],
                            QT[:, h, tqc * 512 + off:(tqc + 1) * 512],
                            start=True,
                            stop=True,
                        )
                        es = work.tile([P, 512], bf16, tag="es", name="es",
                                       bufs=6)
                        nc.scalar.activation(
                            es[:, 0:w], pss[:, 0:w],
                            mybir.ActivationFunctionType.Exp,
                            scale=SCALE,
                        )
                        if i >= 0:
                            nc.vector.tensor_tensor(
                                out=es[:, 0:P], in0=es[:, 0:P], in1=tri[:],
                                op=mybir.AluOpType.mult,
                            )
                        return es

                    psy = ps2.tile([P, 512], f32, tag="psy", name="psy",
                                   bufs=2)
                    dA = work.tile([P, 512], fp16, tag="dA", name="dA",
                                   bufs=2)
                    dB = None
                    if g > 0:
                        dB = work.tile([P, 512], fp16, tag="dB", name="dB",
                                       bufs=2)

                    pipe = {}
                    for tk in range(min(ahead, ntk)):
                        pipe[tk] = scores_exp(tk)
                    for tk in range(ntk):
                        if tk + ahead < ntk:
                            pipe[tk + ahead] = scores_exp(tk + ahead)
                        i = tk - 4 * tqc
                        off = max(0, i) * P
                        w = 512 - off
                        es = pipe.pop(tk)
                        # AV: y^T[d, tq] += V_nat[tk].T @ S^T[tk, tq]
                        nc.tensor.matmul(
                            psy[:, off:512],
                            VN[:, tk],
                            es[:, 0:w],
                            start=(tk == 0),
                            stop=(tk == ntk - 1),
                        )
                        # Denominator partials: d[p, tq] += es[p, tq]
                        if tk < g:
                            eng, acc, first = nc.gpsimd, dB, tk == 0
                        else:
                            eng, acc, first = nc.vector, dA, tk == g
                        if first:
                            eng.tensor_copy(acc[:, off:512], es[:, 0:w])
                        else:
                            eng.tensor_tensor(
                                out=acc[:, off:512], in0=acc[:, off:512],
                                in1=es[:, 0:w], op=mybir.AluOpType.add,
                            )

                    # den[tq] = sum_p d[p, tq], broadcast to all partitions
                    # by the ones-stationary matmul; then y^T / den fused
                    # into the PSUM evacuation.
                    psden = ps2.tile([P, 512], f32, tag="psden", name="psden",
                                     bufs=1)
                    nc.tensor.matmul(psden[:], ones16[:], dA[:],
                                     start=True, stop=(g == 0))
                    if g > 0:
                        nc.tensor.matmul(psden[:], ones16[:], dB[:],
                                         start=False, stop=True)
                    rden = work.tile([P, 512], f32, tag="rden", name="rden",
                                     bufs=2)
                    nc.vector.reciprocal(rden[:], psden[:])
                    nc.vector.tensor_tensor(
                        out=YT[:, h, tqc * 512:(tqc + 1) * 512],
                        in0=psy[:], in1=rden[:],
                        op=mybir.AluOpType.mult,
                    )

                if tqc > 0 and do_oproj:
                    oproj_chunk(tqc - 1)
            if do_attn and do_oproj:
                oproj_chunk(TQC - 1)


def _get_nc():
    if "nc" not in _NC_CACHE:
        _NC_CACHE["nc"] = _build_nc()
    return _NC_CACHE["nc"]


def _tile_e(arr, chunk):
    # [out_dim, E] -> transpose -> [E, out_dim] -> [128, KO, out_dim]
    import ml_dtypes

    t = np.ascontiguousarray(arr.T)
    t = t.reshape(KO, P, chunk).transpose(1, 0, 2)
    return np.ascontiguousarray(t).astype(ml_dtypes.bfloat16)


def _in_maps(x, wq, wk, wv, wo):
    import ml_dtypes

    bf16 = ml_dtypes.bfloat16
    xT3 = [_tile_e(x[b], T) for b in range(B)]
    wqT3 = [_tile_e(wq[g * 512:(g + 1) * 512], 512) for g in range(NKV)]
    wkT3 = [_tile_e(wk[g * HD:(g + 1) * HD], HD) for g in range(NKV)]
    wvT3 = [_tile_e(wv[g * HD:(g + 1) * HD], HD) for g in range(NKV)]
    # wo columns for group g, transposed to [512, E] then tiled to [128,4,E]
    woT3 = []
    for g in range(NKV):
        t = np.ascontiguousarray(wo[:, g * 512:(g + 1) * 512].T)  # [512, E]
        t = t.reshape(NHC, P, E).transpose(1, 0, 2)
        woT3.append(np.ascontiguousarray(t).astype(bf16))
    maps = []
    for c in range(N_CORES):
        b, g = divmod(c, NKV)
        maps.append({
            "xT3": xT3[b],
            "wqT3": wqT3[g],
            "wkT3": wkT3[g],
            "wvT3": wvT3[g],
            "woT3": woT3[g],
        })
    return maps


def kernel(x, wq, wk, wv, wo):
    from concourse.bass_utils import run_bass_kernel_spmd

    x = np.asarray(x, dtype=np.float32)
    wq = np.asarray(wq, dtype=np.float32)
    wk = np.asarray(wk, dtype=np.float32)
    wv = np.asarray(wv, dtype=np.float32)
    wo = np.asarray(wo, dtype=np.float32)

    nc = _get_nc()
    in_maps = _in_maps(x, wq, wk, wv, wo)

    res = run_bass_kernel_spmd(nc, in_maps, core_ids=list(range(N_CORES)))

    partials = [np.asarray(res.results[c]["out"]).astype(np.float32)
                for c in range(N_CORES)]
    out = np.empty((B, T, E), dtype=np.float32)
    for b in range(B):
        acc = partials[NKV * b]
        for g in range(1, NKV):
            acc = acc + partials[NKV * b + g]
        out[b] = acc
    return out
